# revision 1
# baseline (speedup 1.0000x reference)
"""DistSAGE 3-layer GraphSAGE forward on 8 TRN2 NeuronCores (Bass/Tile).

Strategy (graph/data parallel, per the DistSAGE recipe):
  - Partition the 512 seed nodes across 8 cores (64 each, LPT-balanced by
    an additive 2-hop cost estimate); build per-core dependency-driven
    blocks on the host (standard DGL block construction, pure index math).
    No inter-core communication; weights replicated.
  - Row-shard the feature table: each core receives a compact bf16 table
    holding only the x rows its block touches, organized as per-dst-tile
    bands [128 h_dst rows | the tile's unique source rows], stored
    PRE-INTERLEAVED in SBUF layout (row g*128+p at [partition p, group g])
    so every layer-0 tile's working set loads as one contiguous dense DMA
    at line rate -- no per-row gather descriptors at all.
  - Per 128-dst tile: meanT[f,d] += msgs_chunk.T @ S' accumulates on the
    TensorEngine in PSUM, where S'[p,d] = sum of 1/deg[d] over edges
    (band_row -> d) is a host-baked bf16 mask streamed on the second HWDGE
    ring; h_dstT comes from 2 identity matmuls of the band's dst group.
    Then Y[d,:] = meanT.T@W_neigh + h_dstT.T@W_self + 1s^T@bias
    (PSUM-accumulated bf16 matmuls), ReLU straight from PSUM, DMA the
    bf16 tile to DRAM.
  - Layer 1 reads h1 (DRAM) with the dma_gather ucode, but the l0 rows are
    CONSUMER-GROUPED ([l1_out | multi-tile srcs | per-l1-tile source bands])
    and each gather call carries [lo, hi) row bounds on its in_ap, so calls
    only depend on the h1 rows they actually read -> the gathers stream in
    under layer-0 compute instead of serializing at the layer boundary.
    Calls are tile-aligned (big calls + a 2-chunk tail per tile), emitted in
    readiness (hi) order after the layer-0 loop (PE queues are in-order, so
    layer-1 matmuls must not be emitted mid-stream or they head-of-line
    block layer 0 on gather latency).
  - Layer 2 is a dense mask-matmul sweep over SBUF-resident h2 tiles
    (host-baked [128, T1*128] masks) -- no gather, no h2 DRAM roundtrip.
  - DMA triggers are spread across Sync (band + sp0 streams) and Scalar
    (h1 writes + late consts); non-critical consts (weights 1/2, gather
    indices, sp slabs) load behind the first band tile so the main stream
    starts immediately.  The bias matmul is skipped when biases are zero.
"""

import heapq

import numpy as np

P = 128
NCORES = 8
NUM_DST = (61952, 5632, 512)
FEAT = 256
OUTW = (256, 256, 19)
SEEDS_PER_CORE = NUM_DST[2] // NCORES  # 64
WINDOW = 32768
NI_GATHER = 1024  # dma_gather indices per call (layers 1/2)


def _bf16():
    import ml_dtypes

    return ml_dtypes.bfloat16


# ---------------------------------------------------------------------------
# Host-side block construction
# ---------------------------------------------------------------------------


def _balance(ids, deg, n_buckets):
    """LPT bin-packing: reorder ids so consecutive 128-groups have ~equal
    total degree (only full 128-groups are balanced)."""
    if n_buckets <= 1 or len(ids) < n_buckets * P:
        return ids
    order = np.argsort(-deg[ids], kind="stable")
    heap = [(0.0, b, 0) for b in range(n_buckets)]
    heapq.heapify(heap)
    buckets = [[] for _ in range(n_buckets)]
    for i in order:
        load, b, cnt = heapq.heappop(heap)
        buckets[b].append(ids[i])
        cnt += 1
        if cnt < P:
            heapq.heappush(heap, (load + deg[ids[i]], b, cnt))
    return np.concatenate([np.asarray(b, dtype=ids.dtype) for b in buckets])


def _seed_partition(esrc0, edst0, esrc1, edst1, esrc2, edst2, deg0, deg1):
    """LPT-balance seeds across cores by an additive 2-hop cost estimate."""
    h = np.zeros(NUM_DST[1], np.float64)
    np.add.at(h, edst1, deg0[esrc1].astype(np.float64))
    cost = np.zeros(NUM_DST[2], np.float64)
    np.add.at(cost, edst2, h[esrc2] + deg1[esrc2].astype(np.float64))
    order = np.argsort(-cost, kind="stable")
    heap = [(0.0, cc, 0) for cc in range(NCORES)]
    heapq.heapify(heap)
    groups = [[] for _ in range(NCORES)]
    for s in order:
        load, cc, cnt = heapq.heappop(heap)
        groups[cc].append(s)
        cnt += 1
        if cnt < SEEDS_PER_CORE:
            heapq.heappush(heap, (load + cost[s], cc, cnt))
    return [np.array(g, dtype=np.int64) for g in groups]


def _block_for_core(seeds, esrc0, edst0, esrc1, edst1, esrc2, edst2,
                    deg0, deg1, deg2):
    pos2 = np.full(NUM_DST[2], -1, np.int32)
    pos2[seeds] = np.arange(SEEDS_PER_CORE, dtype=np.int32)
    sel2 = pos2[edst2] >= 0
    es2, ed2g = esrc2[sel2], edst2[sel2]
    l1_extra = np.setdiff1d(np.unique(es2), seeds)
    nfull = (len(l1_extra) // P) * P
    if nfull >= P:
        l1_extra = np.concatenate(
            [_balance(l1_extra[:nfull], deg1, nfull // P), l1_extra[nfull:]]
        )
    l1_out = np.concatenate([seeds, l1_extra])
    n1 = len(l1_out)

    pos1 = np.full(NUM_DST[1], -1, np.int32)
    pos1[l1_out] = np.arange(n1, dtype=np.int32)
    sel1 = pos1[edst1] >= 0
    es1, ed1g = esrc1[sel1], edst1[sel1]
    ed1 = pos1[ed1g].astype(np.int64)
    inv1 = (1.0 / np.maximum(deg1[ed1g], 1.0)).astype(np.float32)
    # Consumer-grouped l0_extra ordering: [multi-tile srcs | grp0 | grp1 ...]
    # so each layer-1 dst tile's sources sit in a contiguous band of l0 rows
    # (plus the small early multi/l1_out region) -> its gather calls only
    # depend on an early prefix + its own band of h1, enabling overlap of the
    # layer-1 gather under layer-0 compute.
    n1_tiles = -(-n1 // P)
    mask_x = np.ones(len(es1), bool)
    small = es1 < NUM_DST[1]
    mask_x[small] = pos1[es1[small]] < 0
    pr = np.unique(
        np.stack([es1[mask_x], ed1[mask_x] // P], axis=1), axis=0
    )
    srcs_u, first_idx, cnt = np.unique(
        pr[:, 0], return_index=True, return_counts=True
    )
    multi = srcs_u[cnt > 1]
    segs = [multi]
    single_mask = cnt == 1
    s_srcs = srcs_u[single_mask]
    s_tile = pr[first_idx[single_mask], 1]
    for tt in range(n1_tiles):
        seg = s_srcs[s_tile == tt]
        nfull = (len(seg) // P) * P
        if nfull >= P:
            seg = np.concatenate(
                [_balance(seg[:nfull], deg0, nfull // P), seg[nfull:]]
            )
        segs.append(seg)
    l0_extra = np.concatenate(segs)
    l0_out = np.concatenate([l1_out, l0_extra])
    n0 = len(l0_out)

    pos0 = np.full(NUM_DST[0], -1, np.int32)
    pos0[l0_out] = np.arange(n0, dtype=np.int32)
    sel0 = pos0[edst0] >= 0
    es0, ed0g = esrc0[sel0], edst0[sel0]
    ed0 = pos0[ed0g].astype(np.int64)
    inv0 = (1.0 / np.maximum(deg0[ed0g], 1.0)).astype(np.float32)

    ed2 = pos2[ed2g].astype(np.int64)
    inv2 = (1.0 / np.maximum(deg2[ed2g], 1.0)).astype(np.float32)
    es2l = pos1[es2].astype(np.int64)
    es1l = pos0[es1].astype(np.int64)

    return dict(
        seeds=seeds, l1_out=l1_out, l0_out=l0_out, n1=n1, n0=n0,
        e0=(es0.astype(np.int64), ed0, inv0),
        e1=(es1l, ed1, inv1),
        e2=(es2l, ed2, inv2),
    )


def _group_edges_by_tile(es, ed, inv, n_tiles):
    """Per dst-tile: dedup sources, build the dense S' payload.
    Returns per-tile (unique_srcs sorted, W [n_u, 128] f32)."""
    tile = ed // P
    order = np.argsort(tile, kind="stable")
    es, ed, inv, tile = es[order], ed[order], inv[order], tile[order]
    starts = np.searchsorted(tile, np.arange(n_tiles))
    ends = np.searchsorted(tile, np.arange(n_tiles) + 1)
    out = []
    for t in range(n_tiles):
        s, e = starts[t], ends[t]
        u, ii = np.unique(es[s:e], return_inverse=True)
        W = np.zeros((len(u), P), np.float32)
        np.add.at(W, (ii, ed[s:e] - t * P), inv[s:e])
        out.append((u, W))
    return out


class GatherPlan:
    """Layers 1/2: continuous slot stream gathered via dma_gather.
    Tile t owns stream slots [slot_off[t], slot_off[t]+m[t]); chunks are
    128-slot groups; a chunk overlapping two tiles gets one masked S'
    column per tile.  Calls are tile-aligned (big calls + a small tail call
    per tile) and carry [lo, hi) row bounds so each call only depends on the
    h-table rows it reads."""

    def __init__(self, n_tiles, slot_counts, ni):
        self.ni = ni
        self.cpc = ni // P  # max chunks per big call
        self.n_tiles = n_tiles
        self.m = slot_counts
        self.slot_off = np.concatenate([[0], np.cumsum(slot_counts)]).astype(np.int64)
        total = int(self.slot_off[-1])
        self.n_chunks = -(-total // P)
        self.n_chunks_pad = self.n_chunks
        # tile-aligned call partition: big calls + small tail call per tile
        TAILC = 2
        self.call_sizes = []
        for t in range(n_tiles):
            s = -(-int(self.slot_off[t]) // P)
            e = -(-int(self.slot_off[t + 1]) // P) if t + 1 < n_tiles else self.n_chunks
            if t + 1 == n_tiles:
                e = self.n_chunks
            nch = e - s
            if nch <= 0:
                continue
            if nch > TAILC + 1:
                head = nch - TAILC
                nbig = -(-head // self.cpc)
                base = head // nbig
                rem = head - base * nbig
                self.call_sizes += [base + (1 if i < rem else 0)
                                    for i in range(nbig)]
                self.call_sizes.append(TAILC)
            else:
                self.call_sizes.append(nch)
        assert sum(self.call_sizes) == self.n_chunks
        self.n_calls = len(self.call_sizes)
        self.call_chunk_off = np.concatenate(
            [[0], np.cumsum(self.call_sizes)]
        ).astype(np.int64)
        self.pairs = []
        self.tile_pairs = []  # per tile: list of (sp_col, chunk)
        for t in range(n_tiles):
            lo, hi = int(self.slot_off[t]), int(self.slot_off[t + 1])
            ch1 = (hi - 1) // P if hi > lo else lo // P
            tp = []
            for ch in range(lo // P, ch1 + 1):
                tp.append((len(self.pairs), ch))
                self.pairs.append((t, ch))
            self.tile_pairs.append(tp)
        self.n_sp_cols = len(self.pairs)
        self.gidx = []  # [NCORES][128, n_chunks_pad] int64 table rows
        self.wmat = []  # [NCORES][128, n_sp_cols, 128] f32
        self.call_base = None  # [n_calls] row base (lo) per call
        self.call_hi = None  # [n_calls] exclusive row bound per call

    def compute_call_bounds(self, nrows):
        """Per-call [lo, hi) over all cores, 128-aligned."""
        lo = np.zeros(self.n_calls, np.int64)
        hi = np.zeros(self.n_calls, np.int64)
        for k in range(self.n_calls):
            a, b = int(self.call_chunk_off[k]), int(self.call_chunk_off[k + 1])
            mn, mx = nrows, 0
            for g in self.gidx:
                sl = g[:, a:b]
                mn = min(mn, int(sl.min()))
                mx = max(mx, int(sl.max()))
            lo[k] = (mn // P) * P
            hi[k] = min(nrows, ((mx // P) + 1) * P)
        self.call_base = lo
        self.call_hi = hi


def _plan_gather(per_core_tiles, n_tiles, ni):
    m = [
        max(1, max(len(per_core_tiles[c][t][0]) for c in range(NCORES)))
        for t in range(n_tiles)
    ]
    return GatherPlan(n_tiles, m, ni)


def _fill_gather(plan, per_core_tiles, pad_row):
    total_pad = plan.n_chunks_pad * P
    for c in range(NCORES):
        stream = np.zeros(total_pad, np.int64)
        for t in range(plan.n_tiles):
            lo, hi = int(plan.slot_off[t]), int(plan.slot_off[t + 1])
            u, _ = per_core_tiles[c][t]
            stream[lo : lo + len(u)] = u
            stream[lo + len(u) : hi] = pad_row[c][t]
        tail = int(plan.slot_off[-1])
        stream[tail:] = pad_row[c][plan.n_tiles - 1]
        plan.gidx.append(stream.reshape(plan.n_chunks_pad, P).T.copy())

        wmat = np.zeros((P, plan.n_sp_cols, P), np.float32)
        for t in range(plan.n_tiles):
            lo = int(plan.slot_off[t])
            u, W = per_core_tiles[c][t]
            for sp_col, ch in plan.tile_pairs[t]:
                s0 = ch * P
                a = max(s0, lo)
                b = min(s0 + P, lo + len(u))
                if a < b:
                    wmat[a - s0 : b - s0, sp_col, :] = W[a - lo : b - lo]
        plan.wmat.append(wmat)


class BandPlan:
    """Layer 0: per-tile dense bands, pre-interleaved.  Tile t's band =
    group 0 (h_dst rows) + groups 1..K[t] (source chunks); group g sits at
    xc2[:, (goff[t]+g)*256 : ...]."""

    def __init__(self, n_tiles, src_counts):
        self.n_tiles = n_tiles
        self.m = src_counts  # real (max-over-core) source count per tile
        self.K = [max(1, -(-m // P)) for m in src_counts]
        self.goff = np.concatenate(
            [[0], np.cumsum([1 + k for k in self.K])]
        ).astype(np.int64)
        self.n_groups = int(self.goff[-1])
        self.n_sp_cols = sum(self.K)
        self.sp_off = np.concatenate([[0], np.cumsum(self.K)]).astype(np.int64)
        self.wmat = []  # [NCORES][128, n_sp_cols, 128] f32


def build_host(inputs):
    esrc0 = np.asarray(inputs["esrc0"]).astype(np.int64)
    edst0 = np.asarray(inputs["edst0"]).astype(np.int64)
    esrc1 = np.asarray(inputs["esrc1"]).astype(np.int64)
    edst1 = np.asarray(inputs["edst1"]).astype(np.int64)
    esrc2 = np.asarray(inputs["esrc2"]).astype(np.int64)
    edst2 = np.asarray(inputs["edst2"]).astype(np.int64)
    x = np.asarray(inputs["x"], dtype=np.float32)

    deg0 = np.bincount(edst0, minlength=NUM_DST[0]).astype(np.float32)
    deg1 = np.bincount(edst1, minlength=NUM_DST[1]).astype(np.float32)
    deg2 = np.bincount(edst2, minlength=NUM_DST[2]).astype(np.float32)

    seed_groups = _seed_partition(esrc0, edst0, esrc1, edst1, esrc2, edst2,
                                  deg0, deg1)
    blocks = [
        _block_for_core(seed_groups[c], esrc0, edst0, esrc1, edst1, esrc2,
                        edst2, deg0, deg1, deg2)
        for c in range(NCORES)
    ]

    n0_pad = max(-(-b["n0"] // P) for b in blocks) * P
    n1_pad = max(-(-b["n1"] // P) for b in blocks) * P
    T0, T1, T2 = n0_pad // P, n1_pad // P, 1

    tiles0 = [_group_edges_by_tile(*b["e0"], T0) for b in blocks]
    tiles1 = [_group_edges_by_tile(*b["e1"], T1) for b in blocks]

    # ---- layer 0: band plan + pre-interleaved compact tables ----
    plan0 = BandPlan(
        T0,
        [max(len(tiles0[c][t][0]) for c in range(NCORES)) for t in range(T0)],
    )
    l0_padded = []
    for b in blocks:
        v = np.zeros(T0 * P, np.int64)
        v[: b["n0"]] = b["l0_out"]
        v[b["n0"] :] = b["l0_out"][0]
        l0_padded.append(v)

    bf16 = _bf16()
    x16 = x.astype(bf16)
    xc2s = []
    for c in range(NCORES):
        xr = np.zeros((P, plan0.n_groups, FEAT), bf16)
        wmat = np.zeros((P, plan0.n_sp_cols, P), np.float32)
        for t in range(T0):
            g0 = int(plan0.goff[t])
            xr[:, g0, :] = x16[l0_padded[c][t * P : (t + 1) * P]]
            u, W = tiles0[c][t]
            rows = x16[u]
            for k in range(plan0.K[t]):
                a, b = k * P, min((k + 1) * P, len(u))
                if a < b:
                    xr[: b - a, g0 + 1 + k, :] = rows[a:b]
                    wmat[: b - a, int(plan0.sp_off[t]) + k, :] = W[a:b]
        xc2s.append(np.ascontiguousarray(xr.reshape(P, plan0.n_groups * FEAT)))
        plan0.wmat.append(wmat)

    # ---- layer 1: gather plan ----
    plan1 = _plan_gather(tiles1, T1, NI_GATHER)
    padL = lambda T: [[t * P for t in range(T)] for _ in range(NCORES)]
    _fill_gather(plan1, tiles1, padL(T1))
    plan1.compute_call_bounds(n0_pad)
    assert n0_pad <= WINDOW and n1_pad <= WINDOW

    # ---- layer 2: dense sweep over SBUF-resident h2 (no gather) ----
    # sp2dense[c][j] = [128, 128] mask: W[row, seed] = sum inv2 over edges
    # (src local j*128+row -> seed).
    sp2d = []
    for c in range(NCORES):
        es, ed, inv = blocks[c]["e2"]
        W = np.zeros((T1, P, P), np.float32)
        np.add.at(W, (es // P, es % P, ed), inv)
        sp2d.append(
            np.ascontiguousarray(
                W.transpose(1, 0, 2).reshape(P, T1 * P).astype(bf16)
            )
        )

    return dict(
        plan0=plan0,
        plans=(plan1,),
        sp2d=sp2d,
        T=(T0, T1, T2),
        n0_pad=n0_pad,
        n1_pad=n1_pad,
        xc2s=xc2s,
        blocks=blocks,
        weights=tuple(
            (
                np.asarray(inputs[f"W_self{l}"], np.float32),
                np.asarray(inputs[f"W_neigh{l}"], np.float32),
                np.asarray(inputs[f"b{l}"], np.float32),
            )
            for l in range(3)
        ),
    )


# ---------------------------------------------------------------------------
# Numpy simulation of the device kernel (validation aid; fp32 stand-in)
# ---------------------------------------------------------------------------


def simulate_core(meta, c):
    plan0 = meta["plan0"]
    xr = meta["xc2s"][c].astype(np.float32).reshape(P, plan0.n_groups, FEAT)

    ws, wn, b = meta["weights"][0]
    table = np.zeros((plan0.n_tiles * P, OUTW[0]), np.float32)
    for t in range(plan0.n_tiles):
        g0 = int(plan0.goff[t])
        hd = xr[:, g0, :]
        aggT = np.zeros((FEAT, P), np.float32)
        for k in range(plan0.K[t]):
            msgs = xr[:, g0 + 1 + k, :]
            aggT += msgs.T @ plan0.wmat[c][:, int(plan0.sp_off[t]) + k, :]
        table[t * P : (t + 1) * P] = np.maximum(hd @ ws + aggT.T @ wn + b, 0.0)

    plan = meta["plans"][0]
    ws, wn, b = meta["weights"][1]
    out = np.zeros((plan.n_tiles * P, OUTW[1]), np.float32)
    for t in range(plan.n_tiles):
        hd = table[t * P : (t + 1) * P]
        aggT = np.zeros((FEAT, P), np.float32)
        for sp_col, ch in plan.tile_pairs[t]:
            msgs = table[plan.gidx[c][:, ch]]
            aggT += msgs.T @ plan.wmat[c][:, sp_col, :]
        out[t * P : (t + 1) * P] = np.maximum(hd @ ws + aggT.T @ wn + b, 0.0)
    table = out

    # layer 2: dense sweep
    ws, wn, b = meta["weights"][2]
    sp2 = meta["sp2d"][c].astype(np.float32).reshape(P, -1, P)
    hd = table[0:P]
    aggT = np.zeros((FEAT, P), np.float32)
    for j in range(sp2.shape[1]):
        aggT += table[j * P : (j + 1) * P].T @ sp2[:, j, :]
    y = hd @ ws + aggT.T @ wn + b
    return y[:SEEDS_PER_CORE]


# ---------------------------------------------------------------------------
# Device kernel
# ---------------------------------------------------------------------------


def _wrap_idx16(plan, c):
    bases = np.zeros(plan.n_chunks_pad, np.int64)
    for k in range(plan.n_calls):
        bases[plan.call_chunk_off[k] : plan.call_chunk_off[k + 1]] = plan.call_base[k]
    rel = plan.gidx[c] - bases[None, :]
    total16 = plan.n_chunks_pad * P // 16
    out = np.zeros((P, total16), np.int16)
    off16 = 0
    for k in range(plan.n_calls):
        a, b = int(plan.call_chunk_off[k]), int(plan.call_chunk_off[k + 1])
        flat = rel[:, a:b].T.reshape(-1)
        w = flat.reshape(len(flat) // 16, 16).T.astype(np.int16)
        out[:16, off16 : off16 + w.shape[1]] = w
        off16 += w.shape[1]
    for rep in range(1, 8):
        out[rep * 16 : (rep + 1) * 16] = out[:16]
    return out


def run_device(meta, trace=False):
    import concourse.bacc as bacc
    import concourse.tile as tile
    import concourse.mybir as mybir
    from concourse.bass_utils import run_bass_kernel_spmd

    plan0 = meta["plan0"]
    plan1 = meta["plans"][0]
    T1 = meta["T"][1]
    f32 = mybir.dt.float32
    b16 = mybir.dt.bfloat16

    nc = bacc.Bacc("TRN2", target_bir_lowering=False, debug=False, num_devices=NCORES)

    xc2 = nc.dram_tensor("xc2", [P, plan0.n_groups * FEAT], b16, kind="ExternalInput")
    sp0_d = nc.dram_tensor("sp0", [P, plan0.n_sp_cols * P], b16, kind="ExternalInput")
    ident_d = nc.dram_tensor("ident", [P, P], b16, kind="ExternalInput")
    ones_d = nc.dram_tensor("ones", [1, P], b16, kind="ExternalInput")
    h1buf = nc.dram_tensor("h1buf", [meta["n0_pad"], FEAT], b16)
    out_d = nc.dram_tensor("out", [SEEDS_PER_CORE, OUTW[2]], f32, kind="ExternalOutput")

    idx1_d = nc.dram_tensor("gidx1", [P, plan1.n_chunks_pad * P // 16],
                            mybir.dt.int16, kind="ExternalInput")
    sp1_d = nc.dram_tensor("sp1", [P, plan1.n_sp_cols * P], b16,
                           kind="ExternalInput")
    sp2_d = nc.dram_tensor("sp2d", [P, T1 * P], b16, kind="ExternalInput")
    w_d = []
    for l in range(3):
        w_d.append(
            (
                nc.dram_tensor(f"ws{l}", [FEAT, OUTW[l]], b16, kind="ExternalInput"),
                nc.dram_tensor(f"wn{l}", [FEAT, OUTW[l]], b16, kind="ExternalInput"),
                nc.dram_tensor(f"bias{l}", [1, OUTW[l]], b16, kind="ExternalInput"),
            )
        )

    use_bias = [bool(np.any(meta["weights"][l][2] != 0)) for l in range(3)]

    with tile.TileContext(nc) as tc:
        with (
            tc.tile_pool(name="const", bufs=1) as cpool,
            tc.tile_pool(name="msgs", bufs=6) as mpool,
            tc.tile_pool(name="sel", bufs=6) as spool,
            tc.tile_pool(name="acc", bufs=2) as apool,
            tc.tile_pool(name="outp", bufs=3) as opool,
            tc.tile_pool(name="gmsg", bufs=1) as gpool,
            tc.tile_pool(name="pagg", bufs=2, space="PSUM") as pa,
            tc.tile_pool(name="py", bufs=2, space="PSUM") as pypool,
        ):
            # ---- minimal upfront consts (keep the band stream unblocked) ----
            ident_t = cpool.tile([P, P], b16, tag="ident")
            nc.scalar.dma_start(out=ident_t[:], in_=ident_d[:])
            ws_ts, wn_ts, bias_ts = [[None, None] for _ in range(3)], \
                [[None, None] for _ in range(3)], [None] * 3
            ones_t = cpool.tile([1, P], b16, tag="ones")

            def load_weights(l, eng):
                outw = OUTW[l]
                for k in range(2):
                    w = cpool.tile([P, outw], b16, tag=f"ws{l}_{k}")
                    eng.dma_start(out=w[:], in_=w_d[l][0][k * P : (k + 1) * P, :])
                    ws_ts[l][k] = w
                    w = cpool.tile([P, outw], b16, tag=f"wn{l}_{k}")
                    eng.dma_start(out=w[:], in_=w_d[l][1][k * P : (k + 1) * P, :])
                    wn_ts[l][k] = w
                if use_bias[l]:
                    bias_t = cpool.tile([1, outw], b16, tag=f"bias{l}")
                    eng.dma_start(out=bias_t[:], in_=w_d[l][2][:])
                    bias_ts[l] = bias_t

            load_weights(0, nc.scalar)
            if any(use_bias):
                nc.scalar.dma_start(out=ones_t[:], in_=ones_d[:])

            h2res = [
                cpool.tile([P, FEAT], b16, tag=f"h2res_{t}", name=f"h2res_{t}")
                for t in range(T1)
            ]

            def tile_tail(l, t, ac0, ac1, dest):
                """Y matmuls + bias + activation + store for one dst tile."""
                outw = OUTW[l]
                y = pypool.tile([P, outw], f32, tag="y")
                nc.tensor.matmul(y[:], lhsT=ac0[:, 0:P], rhs=wn_ts[l][0][:],
                                 start=True, stop=False)
                nc.tensor.matmul(y[:], lhsT=ac1[:, 0:P], rhs=wn_ts[l][1][:],
                                 start=False, stop=False)
                nc.tensor.matmul(y[:], lhsT=ac0[:, P : 2 * P], rhs=ws_ts[l][0][:],
                                 start=False, stop=False)
                nc.tensor.matmul(y[:], lhsT=ac1[:, P : 2 * P], rhs=ws_ts[l][1][:],
                                 start=False, stop=not use_bias[l])
                if use_bias[l]:
                    nc.tensor.matmul(y[:], lhsT=ones_t[0:1, :],
                                     rhs=bias_ts[l][0:1, :],
                                     start=False, stop=True)
                if l == 0:
                    o2 = opool.tile([P, outw], b16, tag="o2")
                    nc.scalar.activation(
                        out=o2[:], in_=y[:],
                        func=mybir.ActivationFunctionType.Relu,
                    )
                    nc.scalar.dma_start(out=dest[t * P : (t + 1) * P, :], in_=o2[:])
                elif l == 1:
                    nc.scalar.activation(
                        out=h2res[t][:], in_=y[:],
                        func=mybir.ActivationFunctionType.Relu,
                    )
                else:
                    o = opool.tile([P, outw], f32, tag="o")
                    nc.vector.tensor_copy(out=o[:], in_=y[:])
                    nc.sync.dma_start(out=dest[:], in_=o[0:SEEDS_PER_CORE, :])

            # ================= layer 0: dense bands =================
            Kmax = max(plan0.K)
            for t in range(plan0.n_tiles):
                K = plan0.K[t]
                g0 = int(plan0.goff[t])
                bt = mpool.tile([P, (1 + Kmax) * FEAT], b16, tag="band")
                nc.sync.dma_start(
                    out=bt[:, : (1 + K) * FEAT],
                    in_=xc2[:, g0 * FEAT : (g0 + 1 + K) * FEAT],
                )
                spt = spool.tile([P, Kmax * P], b16, tag="spb")
                so = int(plan0.sp_off[t])
                nc.sync.dma_start(
                    out=spt[:, : K * P], in_=sp0_d[:, so * P : (so + K) * P]
                )
                pc0 = pa.tile([P, 2 * P], f32, tag="pc0")
                pc1 = pa.tile([P, 2 * P], f32, tag="pc1")
                nc.tensor.matmul(pc0[:, P : 2 * P], lhsT=bt[:, 0:P],
                                 rhs=ident_t[:], start=True, stop=True)
                nc.tensor.matmul(pc1[:, P : 2 * P], lhsT=bt[:, P : 2 * P],
                                 rhs=ident_t[:], start=True, stop=True)
                for k in range(K):
                    st, sp = (k == 0), (k == K - 1)
                    base = (1 + k) * FEAT
                    nc.tensor.matmul(pc0[:, 0:P], lhsT=bt[:, base : base + P],
                                     rhs=spt[:, k * P : (k + 1) * P],
                                     start=st, stop=sp)
                    nc.tensor.matmul(pc1[:, 0:P],
                                     lhsT=bt[:, base + P : base + 2 * P],
                                     rhs=spt[:, k * P : (k + 1) * P],
                                     start=st, stop=sp)
                ac0 = apool.tile([P, 2 * P], b16, tag="ac0")
                nc.vector.tensor_copy(out=ac0[:], in_=pc0[:])
                ac1 = apool.tile([P, 2 * P], b16, tag="ac1")
                nc.vector.tensor_copy(out=ac1[:], in_=pc1[:])
                tile_tail(0, t, ac0, ac1, h1buf)

                if t == 0:
                    # late consts: emitted behind the first band loads so the
                    # main stream starts immediately; all are ready long
                    # before their consumers run.
                    idx1_t = cpool.tile(list(idx1_d.shape), mybir.dt.int16,
                                        tag="idx1")
                    nc.scalar.dma_start(out=idx1_t[:], in_=idx1_d[:])
                    load_weights(1, nc.scalar)
                    load_weights(2, nc.scalar)
                    sp2_t = cpool.tile([P, T1 * P], b16, tag="sp2d")
                    nc.scalar.dma_start(out=sp2_t[:], in_=sp2_d[:])
                    SPG = 16
                    sp1_tiles = []
                    n_slabs = -(-plan1.n_sp_cols // SPG)
                    for k in range(n_slabs):
                        c0 = k * SPG * P
                        c1 = min((k + 1) * SPG * P, plan1.n_sp_cols * P)
                        st = cpool.tile([P, c1 - c0], b16, tag=f"sp1_{k}")
                        nc.scalar.dma_start(out=st[:], in_=sp1_d[:, c0:c1])
                        sp1_tiles.append(st)

            # ================= layer 1: overlapped gather =================
            call_tiles = [None] * plan1.n_calls
            order = sorted(
                range(plan1.n_calls),
                key=lambda k: (int(plan1.call_hi[k]), int(plan1.call_base[k])),
            )
            for k in order:
                a = int(plan1.call_chunk_off[k])
                b2 = int(plan1.call_chunk_off[k + 1])
                sz = b2 - a
                lo = int(plan1.call_base[k])
                hi = int(plan1.call_hi[k])
                mt = gpool.tile([P, sz * FEAT], b16, tag=f"msgs1_{k}")
                nc.gpsimd.dma_gather(
                    out_ap=mt[:, : sz * FEAT].rearrange("p (g d) -> p g d", g=sz),
                    in_ap=h1buf[lo:hi, :],
                    idxs_ap=idx1_t[:, a * P // 16 : b2 * P // 16],
                    num_idxs=sz * P,
                    num_idxs_reg=sz * P,
                    elem_size=FEAT,
                    single_packet=False,
                )
                call_tiles[k] = (mt, a)

            call_of_chunk = np.searchsorted(
                plan1.call_chunk_off, np.arange(plan1.n_chunks_pad), side="right"
            ) - 1

            def msg_slice(ch, f0, f1):
                k = int(call_of_chunk[ch])
                mt, a = call_tiles[k]
                j = ch - a
                return mt[:, j * FEAT + f0 : j * FEAT + f1]

            def sp_slice(col):
                k, j = divmod(col, 16)
                return sp1_tiles[k][:, j * P : (j + 1) * P]

            for t in range(plan1.n_tiles):
                hd = opool.tile([P, FEAT], b16, tag="hd")
                nc.sync.dma_start(out=hd[:], in_=h1buf[t * P : (t + 1) * P, :])
                pc0 = pa.tile([P, 2 * P], f32, tag="pc0")
                pc1 = pa.tile([P, 2 * P], f32, tag="pc1")
                nc.tensor.matmul(pc0[:, P : 2 * P], lhsT=hd[:, 0:P],
                                 rhs=ident_t[:], start=True, stop=True)
                nc.tensor.matmul(pc1[:, P : 2 * P], lhsT=hd[:, P : 2 * P],
                                 rhs=ident_t[:], start=True, stop=True)
                pairs = plan1.tile_pairs[t]
                for i, (sp_col, ch) in enumerate(pairs):
                    st, sp = (i == 0), (i == len(pairs) - 1)
                    nc.tensor.matmul(pc0[:, 0:P], lhsT=msg_slice(ch, 0, P),
                                     rhs=sp_slice(sp_col), start=st, stop=sp)
                    nc.tensor.matmul(pc1[:, 0:P], lhsT=msg_slice(ch, P, 2 * P),
                                     rhs=sp_slice(sp_col), start=st, stop=sp)
                ac0 = apool.tile([P, 2 * P], b16, tag="ac0")
                nc.vector.tensor_copy(out=ac0[:], in_=pc0[:])
                ac1 = apool.tile([P, 2 * P], b16, tag="ac1")
                nc.vector.tensor_copy(out=ac1[:], in_=pc1[:])
                tile_tail(1, t, ac0, ac1, None)

            # ================= layer 2: dense sweep over h2res =================
            pc0 = pa.tile([P, 2 * P], f32, tag="pc0")
            pc1 = pa.tile([P, 2 * P], f32, tag="pc1")
            nc.tensor.matmul(pc0[:, P : 2 * P], lhsT=h2res[0][:, 0:P],
                             rhs=ident_t[:], start=True, stop=True)
            nc.tensor.matmul(pc1[:, P : 2 * P], lhsT=h2res[0][:, P : 2 * P],
                             rhs=ident_t[:], start=True, stop=True)
            for j in range(T1):
                st, sp = (j == 0), (j == T1 - 1)
                nc.tensor.matmul(pc0[:, 0:P], lhsT=h2res[j][:, 0:P],
                                 rhs=sp2_t[:, j * P : (j + 1) * P],
                                 start=st, stop=sp)
                nc.tensor.matmul(pc1[:, 0:P], lhsT=h2res[j][:, P : 2 * P],
                                 rhs=sp2_t[:, j * P : (j + 1) * P],
                                 start=st, stop=sp)
            ac0 = apool.tile([P, 2 * P], b16, tag="ac0")
            nc.vector.tensor_copy(out=ac0[:], in_=pc0[:])
            ac1 = apool.tile([P, 2 * P], b16, tag="ac1")
            nc.vector.tensor_copy(out=ac1[:], in_=pc1[:])
            tile_tail(2, 0, ac0, ac1, out_d)

    nc.compile()

    in_maps = []
    bf16 = _bf16()
    eye16 = np.eye(P, dtype=bf16)
    for c in range(NCORES):
        m = dict(
            xc2=meta["xc2s"][c],
            sp0=np.ascontiguousarray(
                plan0.wmat[c].astype(bf16).reshape(P, plan0.n_sp_cols * P)
            ),
            ident=eye16,
            ones=np.ones((1, P), dtype=bf16),
            gidx1=_wrap_idx16(plan1, c),
            sp1=np.ascontiguousarray(
                plan1.wmat[c].astype(bf16).reshape(P, plan1.n_sp_cols * P)
            ),
            sp2d=meta["sp2d"][c],
        )
        for l in range(3):
            ws, wn, b = meta["weights"][l]
            m[f"ws{l}"] = np.ascontiguousarray(ws.astype(bf16))
            m[f"wn{l}"] = np.ascontiguousarray(wn.astype(bf16))
            m[f"bias{l}"] = np.ascontiguousarray(b[None, :].astype(bf16))
        in_maps.append(m)

    res = run_bass_kernel_spmd(
        nc, in_maps, core_ids=list(range(NCORES)), trace=trace
    )
    return [res.results[c]["out"] for c in range(NCORES)], res


def assemble(meta, outs):
    full = np.zeros((NUM_DST[2], OUTW[2]), np.float32)
    for c in range(NCORES):
        full[meta["blocks"][c]["seeds"]] = outs[c]
    return full


def kernel(**inputs) -> np.ndarray:
    meta = build_host(inputs)
    outs, _ = run_device(meta)
    return assemble(meta, outs)

